# revision 7
# baseline (speedup 1.0000x reference)
"""Trainium2 Bass kernel for InterpretableMultiHeadAttention.

Sharding: 8 cores, head-parallel. Core c computes heads {2c, 2c+1} for both
batches (q/k/v projections sliced to 128 output features per core), runs full
attention for those heads, then one AllToAll per batch redistributes the
(feature-sharded) attention output into (row-sharded) form so each core
computes the final output projection for 256 rows of each batch.

Key layout/precision choices (driven by the instruction cost model):
  - Activations arrive pre-transposed from host: xT [B, D, S], so projections
    produce feature-major tiles directly.
  - RoPE on DVE in bf16; outputs written to a "folded" fp8 layout
    [32, 2, S] per head (d 0-31 in column block 0, d 32-63 in block 1) so the
    score matmuls can run in fp8 DoubleRow mode (0.5 cycles/row).
  - Scores psum [128 s, 2, 512 t]; exp on the scalar engine over 1024-wide
    tiles, output bf16.
  - AV in [t, d] orientation: accumulators [128 t, 4, 65] packed into one
    PSUM bank (memset + start=False accumulation), ones-column gives the
    softmax denominator; normalization via per-partition reciprocal scalars.
  - Layout transposes (v [d,s]->[s,d] and attention out [t,d]->[d,t]) use the
    DMA XBAR transpose (bf16), costing no PE/DVE/PSUM resources.
  - DMA load is split across the SP, Pool, and (when exp-idle) Activation
    queues; instruction emission interleaves the two batches so projections,
    attention, collectives, and the output projection overlap.
"""

import os
import sys

import numpy as np

sys.path.insert(0, "/opt/trn_rl_repo")

import concourse.bass as bass  # noqa: E402
from concourse import bacc  # noqa: E402
import concourse.tile as tile  # noqa: E402
from concourse import mybir  # noqa: E402

F32 = mybir.dt.float32
F32R = mybir.dt.float32r
BF16 = mybir.dt.bfloat16
FP8 = mybir.dt.float8e4
AF = mybir.ActivationFunctionType
OP = mybir.AluOpType
DR = mybir.MatmulPerfMode.DoubleRow

B = 2
D_MODEL = 1024
NHEAD = 16
HEAD_DIM = 64
N_CORES = 8
P = 128
ROPE_BASE = 10000.0

KC = D_MODEL // P  # 8 contraction chunks for projections
TR = 512  # attention t-range width (one psum pair tile)

# module globals so test.py can flip tracing and read timing
TRACE = bool(int(os.environ.get("BASS_KERNEL_TRACE", "0")))
DEBUG = bool(int(os.environ.get("BASS_KERNEL_DEBUG", "0")))
LAST_RESULTS = None


def r32(ap):
    if ap.dtype == F32R:
        return ap
    return ap.bitcast(F32R)


def build_nc(S=2048, T_TILE=None):
    """Build the SPMD program (identical on all 8 cores)."""
    del T_TILE
    D = D_MODEL
    SC = S // P  # 16 s chunks
    TS = S // N_CORES  # 256 rows per core per batch
    NTR = S // TR  # 4 t-ranges per head
    NTS = TR // P  # 4 t-subchunks per range

    nc = bacc.Bacc()

    xq = nc.declare_dram_parameter("xq", [B, D, S], F32R, isOutput=False)
    xk = nc.declare_dram_parameter("xk", [B, D, S], F32R, isOutput=False)
    xv = nc.declare_dram_parameter("xv", [B, D, S], F32R, isOutput=False)
    wq = nc.declare_dram_parameter("wq", [D, P], F32R, isOutput=False)
    wk = nc.declare_dram_parameter("wk", [D, P], F32R, isOutput=False)
    wv = nc.declare_dram_parameter("wv", [D, P], F32R, isOutput=False)
    bqp = nc.declare_dram_parameter("bq", [P, 1], F32, isOutput=False)
    bkp = nc.declare_dram_parameter("bk", [P, 1], F32, isOutput=False)
    bvp = nc.declare_dram_parameter("bv", [P, 1], F32, isOutput=False)
    wo = nc.declare_dram_parameter("wo", [D, D], F32R, isOutput=False)
    bop = nc.declare_dram_parameter("bo", [P, D], F32, isOutput=False)
    cosr = nc.declare_dram_parameter("cosr", [P, S], BF16, isOutput=False)
    sinr = nc.declare_dram_parameter("sinr", [P, S], BF16, isOutput=False)
    y = nc.declare_dram_parameter("y", [B, TS, D], F32, isOutput=True)
    dbg = {}
    if DEBUG:
        dbg["qfold"] = nc.declare_dram_parameter("dbg_qfold", [64, 2, S], FP8, isOutput=True)
        dbg["kfold"] = nc.declare_dram_parameter("dbg_kfold", [64, 2, S], FP8, isOutput=True)
        dbg["vT"] = nc.declare_dram_parameter("dbg_vT", [P, S], BF16, isOutput=True)
        dbg["v1"] = nc.declare_dram_parameter("dbg_v1", [P, SC, 2, 65], BF16, isOutput=True)
        dbg["pt"] = nc.declare_dram_parameter("dbg_pt", [P, 2, TR], BF16, isOutput=True)
        dbg["dv"] = nc.declare_dram_parameter("dbg_dv", [P, SC, P], BF16, isOutput=True)
        dbg["stg"] = nc.declare_dram_parameter("dbg_stg", [P, SC, P], BF16, isOutput=True)
        dbg["at"] = nc.declare_dram_parameter("dbg_at", [P, KC, TS], BF16, isOutput=True)

    with tile.TileContext(nc) as tc:
        with (
            nc.allow_low_precision(reason="fp8/bf16 attention pipeline"),
            tc.tile_pool(name="singles", bufs=1) as singles,
            tc.tile_pool(name="big", bufs=2) as big,
            tc.tile_pool(name="xt", bufs=4) as xtp,
            tc.tile_pool(name="tmp", bufs=3) as tmp,
            tc.tile_pool(name="pt", bufs=2) as ptp,
            tc.tile_pool(name="io", bufs=2) as iop,
            tc.tile_pool(name="pj", bufs=2, space="PSUM") as pjp,
            tc.tile_pool(name="qcs", bufs=1, space="PSUM") as qcsp,
            tc.tile_pool(name="psc", bufs=2, space="PSUM") as pscp,
            tc.tile_pool(name="pav", bufs=1, space="PSUM") as pavp,
            tc.tile_pool(name="dram", bufs=2, space="DRAM") as dram,
        ):
            # ---- persistent loads (weights etc. go on the ACT queue, which
            # is idle until the first exp) ----
            wq_sb = singles.tile([P, KC, P], F32R, tag="wq")
            wk_sb = singles.tile([P, KC, P], F32R, tag="wk")
            wv_sb = singles.tile([P, KC, P], F32R, tag="wv")
            nc.scalar.dma_start(wk_sb, wk.rearrange("(ko p) m -> p ko m", p=P))
            nc.scalar.dma_start(wv_sb, wv.rearrange("(ko p) m -> p ko m", p=P))
            nc.scalar.dma_start(wq_sb, wq.rearrange("(ko p) m -> p ko m", p=P))
            cos_sb = singles.tile([P, S], BF16, tag="cos")
            sin_sb = singles.tile([P, S], BF16, tag="sin")
            nc.scalar.dma_start(cos_sb, cosr[:, :])
            nc.scalar.dma_start(sin_sb, sinr[:, :])
            bq_sb = singles.tile([P, 1], F32, tag="bq")
            bk_sb = singles.tile([P, 1], F32, tag="bk")
            bv_sb = singles.tile([P, 1], F32, tag="bv")
            nc.sync.dma_start(bq_sb, bqp[:, :])
            nc.sync.dma_start(bk_sb, bkp[:, :])
            nc.sync.dma_start(bv_sb, bvp[:, :])
            bo_sb = singles.tile([P, D], F32, tag="bo")
            nc.scalar.dma_start(bo_sb, bop[:, :])
            wo_sb = singles.tile([P, KC, D], F32R, tag="wo")

            # per-batch state (bufs=2 ping-pongs across batches)
            state = {}

            def proj_stream(b):
                """Projections for batch b. Yields after small quanta."""
                vT = big.tile([P, S], BF16, tag="vT", name="vT")
                qfold = big.tile([64, 2, S], FP8, tag="qfold", name="qfold")
                kfold = big.tile([64, 2, S], FP8, tag="kfold", name="kfold")
                v1 = big.tile([P, SC, 2, 65], BF16, tag="v1", name="v1")
                nc.vector.memset(v1[:, :, :, 64:65], 1.0)
                state[b] = dict(vT=vT, qfold=qfold, kfold=kfold, v1=v1)

                def rope(pj, fold, seg, bias_sb):
                    qb = tmp.tile([P, TR], BF16, tag="qb", name="qb")
                    nc.vector.tensor_scalar_add(qb, pj, bias_sb)
                    qcs = qcsp.tile([P, 2, TR], BF16, tag="qcs", name="qcs")
                    qc = qcs[:, 0, :]
                    qs = qcs[:, 1, :]
                    nc.vector.tensor_tensor(
                        out=qc, in0=qb, in1=cos_sb[:, seg], op=OP.mult
                    )
                    nc.vector.tensor_tensor(
                        out=qs, in0=qb, in1=sin_sb[:, seg], op=OP.mult
                    )
                    for h in range(2):
                        r = 64 * h
                        nc.vector.tensor_tensor(
                            out=fold[32 * h : 32 * h + 32, 0, seg],
                            in0=qc[r : r + 32, :],
                            in1=qs[r + 32 : r + 64, :],
                            op=OP.subtract,
                        )
                        nc.vector.tensor_tensor(
                            out=fold[32 * h : 32 * h + 32, 1, seg],
                            in0=qs[r : r + 32, :],
                            in1=qc[r + 32 : r + 64, :],
                            op=OP.add,
                        )

                # k first, then v, then q: attention needs full k/v but q only
                # tile-by-tile.
                for xin, wsb, bsb, kind in (
                    (xk, wk_sb, bk_sb, "k"),
                    (xv, wv_sb, bv_sb, "v"),
                    (xq, wq_sb, bq_sb, "q"),
                ):
                    for gp in range(S // 1024):  # pairs of 512-col tiles
                        pjs = [
                            pjp.tile([P, TR], F32, tag="pj", name="pj")
                            for _ in range(2)
                        ]
                        engs = (
                            (nc.sync, nc.gpsimd, nc.scalar)
                            if b == 0
                            else (nc.sync, nc.gpsimd)
                        )
                        for k in range(KC):
                            xt_c = xtp.tile([P, 1024], F32R, tag="xt", name="xt")
                            eng = engs[(gp * KC + k) % len(engs)]
                            eng.dma_start(
                                xt_c,
                                xin[b, k * P : (k + 1) * P, gp * 1024 : (gp + 1) * 1024],
                            )
                            for j in range(2):
                                nc.tensor.matmul(
                                    pjs[j],
                                    lhsT=wsb[:, k, :],
                                    rhs=r32(xt_c[:, j * TR : (j + 1) * TR]),
                                    start=(k == 0),
                                    stop=(k == KC - 1),
                                )
                            if k % 4 == 3:
                                yield
                        for j in range(2):
                            g = gp * 2 + j
                            seg = slice(g * TR, (g + 1) * TR)
                            if kind == "v":
                                nc.vector.tensor_scalar_add(vT[:, seg], pjs[j], bsb)
                            elif kind == "q":
                                rope(pjs[j], qfold, seg, bsb)
                            else:
                                rope(pjs[j], kfold, seg, bsb)
                        if kind == "v":
                            # transpose the two finished 512-col blocks into
                            # v1 [s, (h, d)] via DMA XBAR
                            for sc in range(gp * 8, gp * 8 + 8):
                                for h in range(2):
                                    nc.sync.dma_start(
                                        v1[:, sc, h, 0:64],
                                        vT[64 * h : 64 * h + 64, sc * P : (sc + 1) * P],
                                        transpose=True,
                                    )
                        yield

            def attn_stream(b):
                """Attention for batch b (heads sequential, t-ranges of 512)."""
                st = state[b]
                qfold, kfold, v1 = st["qfold"], st["kfold"], st["v1"]
                dv_sb = big.tile([P, SC, P], BF16, tag="dv", name="dv_sb")
                stg = big.tile([P, SC, P], BF16, tag="stg", name="stg")
                state[b]["stg"] = stg
                state[b]["dv_sb"] = dv_sb
                for h in range(2):
                    hp = slice(32 * h, 32 * h + 32)
                    for tr_i in range(NTR):
                        tseg = slice(tr_i * TR, (tr_i + 1) * TR)
                        acc = pavp.tile([P, NTS, P], F32, tag="acc", name="acc")
                        nc.gpsimd.memset(acc[:, :, 0:65], 0.0)
                        for sp in range(SC // 2):
                            psc = pscp.tile([P, 2, TR], F32, tag="psc", name="psc")
                            for j in range(2):
                                sc = 2 * sp + j
                                nc.tensor.matmul(
                                    psc[:, j, :],
                                    lhsT=kfold[hp, :, sc * P : (sc + 1) * P],
                                    rhs=qfold[hp, :, tseg],
                                    start=True,
                                    stop=True,
                                    perf_mode=DR,
                                )
                            pt = ptp.tile([P, 2, TR], BF16, tag="pt", name="pt")
                            nc.scalar.activation(pt, psc, AF.Exp, scale=0.125)
                            if DEBUG and b == 0 and h == 0 and tr_i == 0 and sp == 0:
                                nc.sync.dma_start(dbg["pt"][...], pt[:, :, :])
                            for j in range(2):
                                sc = 2 * sp + j
                                for ts in range(NTS):
                                    nc.tensor.matmul(
                                        acc[:, ts, 0:65],
                                        lhsT=pt[:, j, ts * P : (ts + 1) * P],
                                        rhs=v1[:, sc, h, :],
                                        start=False,
                                        stop=(sp == SC // 2 - 1 and j == 1),
                                        skip_group_check=True,
                                    )
                            yield
                        # epilogue: normalize into dv_sb
                        rc = tmp.tile([P, NTS, 1], F32, tag="rc", name="rc")
                        nc.vector.reciprocal(rc, acc[:, :, 64:65])
                        for ts in range(NTS):
                            nc.gpsimd.tensor_scalar_mul(
                                dv_sb[:, tr_i * NTS + ts, 64 * h : 64 * h + 64],
                                acc[:, ts, 0:64],
                                rc[:, ts, :],
                            )
                        if h == 1:
                            for ts in range(NTS):
                                tsg = tr_i * NTS + ts
                                nc.sync.dma_start(
                                    stg[:, tsg, :],
                                    dv_sb[:, tsg, :],
                                    transpose=True,
                                )
                        yield

            def tail(b, heads=(None,)):
                """Staging write + AllToAll + readback for batch b.

                heads=(None,) does one whole-batch AllToAll; heads=(0,)/(1,)
                do per-head AllToAlls so the first can fire mid-attention.
                """
                stg = state[b]["stg"]
                at_sb = state[b].get("at")
                if at_sb is None:
                    at_sb = iop.tile([P, KC, TS], BF16, tag="at", name="at")
                    state[b]["at"] = at_sb
                for h in heads:
                    hp = slice(0, P) if h is None else slice(64 * h, 64 * h + 64)
                    np_ = P if h is None else 64
                    a2a_in_b = dram.tile(
                        [N_CORES, np_, TS], BF16, tag=f"a2a_in{np_}", name="a2a_in"
                    )
                    a2a_out_b = dram.tile(
                        [N_CORES, np_, TS], BF16, tag=f"a2a_out{np_}", name="a2a_out"
                    )
                    nc.gpsimd.dma_start(
                        a2a_in_b.rearrange("c p t -> p c t"), stg[hp, :, :]
                    )
                    nc.gpsimd.collective_compute(
                        "AllToAll",
                        mybir.AluOpType.bypass,
                        replica_groups=[list(range(N_CORES))],
                        ins=[a2a_in_b.opt()],
                        outs=[a2a_out_b.opt()],
                    )
                    nc.sync.dma_start(
                        at_sb[hp, :, :], a2a_out_b.rearrange("c p t -> p c t")
                    )

            def outproj_stream(b):
                """Output projection for batch b's 256 rows (needs tail(b))."""
                at_sb = state[b]["at"]
                for n in range(D // TR):
                    nseg = slice(n * TR, (n + 1) * TR)
                    for m in range(TS // P):
                        py = pjp.tile([P, TR], F32, tag="pj", name="py")
                        for k in range(KC):
                            nc.tensor.matmul(
                                py,
                                lhsT=at_sb[:, k, m * P : (m + 1) * P],
                                rhs=wo_sb[:, k, nseg],
                                start=(k == 0),
                                stop=(k == KC - 1),
                            )
                        yo = tmp.tile([P, TR], F32, tag="yo", name="yo")
                        nc.vector.tensor_tensor(
                            out=yo, in0=py, in1=bo_sb[:, nseg], op=OP.add
                        )
                        nc.sync.dma_start(y[b, m * P : (m + 1) * P, nseg], yo)
                        yield

            def drain(gen, n=10**9):
                for _ in range(n):
                    try:
                        next(gen)
                    except StopIteration:
                        return True
                return False

            # ---- stage 1: batch-0 projections ----
            p0 = proj_stream(0)
            drain(p0)

            if DEBUG:
                st0 = state[0]
                nc.sync.dma_start(dbg["qfold"][...], st0["qfold"][:, :, :])
                nc.sync.dma_start(dbg["kfold"][...], st0["kfold"][:, :, :])
                nc.sync.dma_start(dbg["vT"][...], st0["vT"][:, :])
                nc.sync.dma_start(dbg["v1"][...], st0["v1"][:, :, :, :])

            # ---- stage 2: batch-0 attention interleaved with batch-1
            # projections ----
            a0 = attn_stream(0)
            p1 = proj_stream(1)
            i = 0
            while True:
                try:
                    next(a0)
                except StopIteration:
                    break
                if i % 2 == 0:
                    drain(p1, 1)
                i += 1
            drain(p1)

            if DEBUG:
                nc.sync.dma_start(dbg["dv"][...], state[0]["dv_sb"][:, :, :])
                nc.sync.dma_start(dbg["stg"][...], state[0]["stg"][:, :, :])

            # ---- stage 3: a2a for b0; batch-1 attention interleaved with
            # wo load + b0 output projection ----
            tail(0)
            if DEBUG:
                nc.sync.dma_start(dbg["at"][...], state[0]["at"][:, :, :])
            a1 = attn_stream(1)
            o0 = outproj_stream(0)
            i = 0
            while True:
                try:
                    next(a1)
                except StopIteration:
                    break
                if i == 2:
                    nc.sync.dma_start(
                        wo_sb, wo.rearrange("(ko p) m -> p ko m", p=P)
                    )
                if i >= 24 and i % 4 == 0:
                    drain(o0, 1)
                i += 1
            drain(o0)

            # ---- stage 4: a2a for b1 + b1 output projection ----
            tail(1)
            o1 = outproj_stream(1)
            drain(o1)

    nc.compile()
    return nc


def host_inputs(query, key_, value, Wq, bq, Wk, bk, Wv, bv, Wo, bo, S=2048):
    """Build per-core input maps (host-side sharding/layout prep)."""
    import ml_dtypes

    f = np.float32
    bf = ml_dtypes.bfloat16
    xq = np.ascontiguousarray(np.transpose(np.asarray(query, f), (0, 2, 1)))
    xk = np.ascontiguousarray(np.transpose(np.asarray(key_, f), (0, 2, 1)))
    xv = np.ascontiguousarray(np.transpose(np.asarray(value, f), (0, 2, 1)))
    wo_t = np.ascontiguousarray(np.asarray(Wo, f).T)
    bo_rep = np.ascontiguousarray(
        np.broadcast_to(np.asarray(bo, f)[None, :], (P, D_MODEL))
    )

    inv_freq = (
        1.0 / (ROPE_BASE ** (np.arange(0, HEAD_DIM, 2, dtype=f) / HEAD_DIM))
    ).astype(f)
    t = np.arange(S, dtype=f)
    freqs = np.einsum("i,j->ij", t, inv_freq).astype(f)  # [S, 32]
    emb = np.concatenate([freqs, freqs], axis=-1)  # [S, 64]
    cosT = np.cos(emb).astype(f).T  # [64, S]
    sinT = np.sin(emb).astype(f).T
    cos_rep = np.ascontiguousarray(np.tile(cosT, (2, 1))).astype(bf)  # [128, S]
    sin_rep = np.ascontiguousarray(np.tile(sinT, (2, 1))).astype(bf)

    Wq, Wk, Wv = (np.asarray(w, f) for w in (Wq, Wk, Wv))
    bq, bk, bv = (np.asarray(v_, f) for v_ in (bq, bk, bv))

    in_maps = []
    for c in range(N_CORES):
        sl = slice(P * c, P * (c + 1))
        in_maps.append(
            {
                "xq": xq,
                "xk": xk,
                "xv": xv,
                "wq": np.ascontiguousarray(Wq[sl, :].T),
                "wk": np.ascontiguousarray(Wk[sl, :].T),
                "wv": np.ascontiguousarray(Wv[sl, :].T),
                "bq": np.ascontiguousarray(bq[sl].reshape(P, 1)),
                "bk": np.ascontiguousarray(bk[sl].reshape(P, 1)),
                "bv": np.ascontiguousarray(bv[sl].reshape(P, 1)),
                "wo": wo_t,
                "bo": bo_rep,
                "cosr": cos_rep,
                "sinr": sin_rep,
            }
        )
    return in_maps


def kernel(query, key_, value, Wq, bq, Wk, bk, Wv, bv, Wo, bo):
    global LAST_RESULTS
    from concourse.bass_utils import run_bass_kernel_spmd

    S = query.shape[1]
    in_maps = host_inputs(
        query, key_, value, Wq, bq, Wk, bk, Wv, bv, Wo, bo, S=S
    )
    nc = build_nc(S=S)
    res = run_bass_kernel_spmd(
        nc, in_maps, core_ids=list(range(N_CORES)), trace=TRACE
    )
    LAST_RESULTS = res
    TS = S // N_CORES
    out = np.empty((B, S, D_MODEL), np.float32)
    for c in range(N_CORES):
        out[:, TS * c : TS * (c + 1), :] = res.results[c]["y"]
    return out


# revision 8
# speedup vs baseline: 1.0577x; 1.0577x over previous
"""Trainium2 Bass kernel for InterpretableMultiHeadAttention.

Sharding: 8 cores, head-parallel. Core c computes heads {2c, 2c+1} for both
batches (q/k/v projections sliced to 128 output features per core), runs full
attention for those heads, then one AllToAll per batch redistributes the
(feature-sharded) attention output into (row-sharded) form so each core
computes the final output projection for 256 rows of each batch.

Key layout/precision choices (driven by the instruction cost model):
  - Activations arrive pre-transposed from host: xT [B, D, S], so projections
    produce feature-major tiles directly.
  - RoPE on DVE in bf16; outputs written to a "folded" fp8 layout
    [32, 2, S] per head (d 0-31 in column block 0, d 32-63 in block 1) so the
    score matmuls can run in fp8 DoubleRow mode (0.5 cycles/row).
  - Scores psum [128 s, 2, 512 t]; exp on the scalar engine over 1024-wide
    tiles, output bf16.
  - AV in [t, d] orientation: accumulators [128 t, 4, 65] packed into one
    PSUM bank (memset + start=False accumulation), ones-column gives the
    softmax denominator; normalization via per-partition reciprocal scalars.
  - Layout transposes (v [d,s]->[s,d] and attention out [t,d]->[d,t]) use the
    DMA XBAR transpose (bf16), costing no PE/DVE/PSUM resources.
  - DMA load is split across the SP, Pool, and (when exp-idle) Activation
    queues; instruction emission interleaves the two batches so projections,
    attention, collectives, and the output projection overlap.
"""

import os
import sys

import numpy as np

sys.path.insert(0, "/opt/trn_rl_repo")

import concourse.bass as bass  # noqa: E402
from concourse import bacc  # noqa: E402
import concourse.tile as tile  # noqa: E402
from concourse import mybir  # noqa: E402

F32 = mybir.dt.float32
F32R = mybir.dt.float32r
BF16 = mybir.dt.bfloat16
FP8 = mybir.dt.float8e4
AF = mybir.ActivationFunctionType
OP = mybir.AluOpType
DR = mybir.MatmulPerfMode.DoubleRow

B = 2
D_MODEL = 1024
NHEAD = 16
HEAD_DIM = 64
N_CORES = 8
P = 128
ROPE_BASE = 10000.0

KC = D_MODEL // P  # 8 contraction chunks for projections
TR = 512  # attention t-range width (one psum pair tile)

# module globals so test.py can flip tracing and read timing
TRACE = bool(int(os.environ.get("BASS_KERNEL_TRACE", "0")))
DEBUG = bool(int(os.environ.get("BASS_KERNEL_DEBUG", "0")))
LAST_RESULTS = None


def r32(ap):
    if ap.dtype == F32R:
        return ap
    return ap.bitcast(F32R)


def build_nc(S=2048, T_TILE=None):
    """Build the SPMD program (identical on all 8 cores)."""
    del T_TILE
    D = D_MODEL
    SC = S // P  # 16 s chunks
    TS = S // N_CORES  # 256 rows per core per batch
    NTR = S // TR  # 4 t-ranges per head
    NTS = TR // P  # 4 t-subchunks per range

    nc = bacc.Bacc()

    xq = nc.declare_dram_parameter("xq", [B, D, S], F32R, isOutput=False)
    xk = nc.declare_dram_parameter("xk", [B, D, S], F32R, isOutput=False)
    xv = nc.declare_dram_parameter("xv", [B, D, S], F32R, isOutput=False)
    wq = nc.declare_dram_parameter("wq", [D, P], F32R, isOutput=False)
    wk = nc.declare_dram_parameter("wk", [D, P], F32R, isOutput=False)
    wv = nc.declare_dram_parameter("wv", [D, P], F32R, isOutput=False)
    bqp = nc.declare_dram_parameter("bq", [P, 1], F32, isOutput=False)
    bkp = nc.declare_dram_parameter("bk", [P, 1], F32, isOutput=False)
    bvp = nc.declare_dram_parameter("bv", [P, 1], F32, isOutput=False)
    wo = nc.declare_dram_parameter("wo", [D, D], F32R, isOutput=False)
    bop = nc.declare_dram_parameter("bo", [P, D], F32, isOutput=False)
    cosr = nc.declare_dram_parameter("cosr", [P, S], BF16, isOutput=False)
    sinr = nc.declare_dram_parameter("sinr", [P, S], BF16, isOutput=False)
    y = nc.declare_dram_parameter("y", [B, TS, D], F32, isOutput=True)
    dbg = {}
    if DEBUG:
        dbg["qfold"] = nc.declare_dram_parameter("dbg_qfold", [64, 2, S], FP8, isOutput=True)
        dbg["kfold"] = nc.declare_dram_parameter("dbg_kfold", [64, 2, S], FP8, isOutput=True)
        dbg["vT"] = nc.declare_dram_parameter("dbg_vT", [P, S], BF16, isOutput=True)
        dbg["v1"] = nc.declare_dram_parameter("dbg_v1", [P, SC, 2, 65], BF16, isOutput=True)
        dbg["pt"] = nc.declare_dram_parameter("dbg_pt", [P, 2, TR], BF16, isOutput=True)
        dbg["dv"] = nc.declare_dram_parameter("dbg_dv", [P, SC, P], BF16, isOutput=True)
        dbg["stg"] = nc.declare_dram_parameter("dbg_stg", [P, SC, P], BF16, isOutput=True)
        dbg["at"] = nc.declare_dram_parameter("dbg_at", [P, KC, TS], BF16, isOutput=True)

    with tile.TileContext(nc) as tc:
        with (
            nc.allow_low_precision(reason="fp8/bf16 attention pipeline"),
            tc.tile_pool(name="singles", bufs=1) as singles,
            tc.tile_pool(name="big", bufs=2) as big,
            tc.tile_pool(name="xt", bufs=8) as xtp,
            tc.tile_pool(name="tmp", bufs=3) as tmp,
            tc.tile_pool(name="pt", bufs=2) as ptp,
            tc.tile_pool(name="io", bufs=2) as iop,
            tc.tile_pool(name="pj", bufs=2, space="PSUM") as pjp,
            tc.tile_pool(name="qcs", bufs=1, space="PSUM") as qcsp,
            tc.tile_pool(name="psc", bufs=2, space="PSUM") as pscp,
            tc.tile_pool(name="pav", bufs=1, space="PSUM") as pavp,
            tc.tile_pool(name="dram", bufs=2, space="DRAM") as dram,
        ):
            # ---- persistent loads (weights etc. go on the ACT queue, which
            # is idle until the first exp) ----
            wq_sb = singles.tile([P, KC, P], F32R, tag="wq")
            wk_sb = singles.tile([P, KC, P], F32R, tag="wk")
            wv_sb = singles.tile([P, KC, P], F32R, tag="wv")
            nc.scalar.dma_start(wk_sb, wk.rearrange("(ko p) m -> p ko m", p=P))
            nc.scalar.dma_start(wv_sb, wv.rearrange("(ko p) m -> p ko m", p=P))
            nc.scalar.dma_start(wq_sb, wq.rearrange("(ko p) m -> p ko m", p=P))
            cos_sb = singles.tile([P, S], BF16, tag="cos")
            sin_sb = singles.tile([P, S], BF16, tag="sin")
            nc.scalar.dma_start(cos_sb, cosr[:, :])
            nc.scalar.dma_start(sin_sb, sinr[:, :])
            bq_sb = singles.tile([P, 1], F32, tag="bq")
            bk_sb = singles.tile([P, 1], F32, tag="bk")
            bv_sb = singles.tile([P, 1], F32, tag="bv")
            nc.sync.dma_start(bq_sb, bqp[:, :])
            nc.sync.dma_start(bk_sb, bkp[:, :])
            nc.sync.dma_start(bv_sb, bvp[:, :])
            bo_sb = singles.tile([P, D], F32, tag="bo")
            nc.scalar.dma_start(bo_sb, bop[:, :])
            wo_sb = singles.tile([P, KC, D], F32R, tag="wo")

            # per-batch state (bufs=2 ping-pongs across batches)
            state = {}

            def proj_stream(b):
                """Projections for batch b. Yields after small quanta."""
                vT = big.tile([P, S], BF16, tag="vT", name="vT")
                qfold = big.tile([64, 2, S], FP8, tag="qfold", name="qfold")
                kfold = big.tile([64, 2, S], FP8, tag="kfold", name="kfold")
                v1 = big.tile([P, SC, 2, 65], BF16, tag="v1", name="v1")
                nc.vector.memset(v1[:, :, :, 64:65], 1.0)
                state[b] = dict(vT=vT, qfold=qfold, kfold=kfold, v1=v1)

                def rope(pj, fold, seg, bias_sb):
                    qb = tmp.tile([P, TR], BF16, tag="qb", name="qb")
                    nc.vector.tensor_scalar_add(qb, pj, bias_sb)
                    qcs = qcsp.tile([P, 2, TR], BF16, tag="qcs", name="qcs")
                    qc = qcs[:, 0, :]
                    qs = qcs[:, 1, :]
                    nc.vector.tensor_tensor(
                        out=qc, in0=qb, in1=cos_sb[:, seg], op=OP.mult
                    )
                    nc.vector.tensor_tensor(
                        out=qs, in0=qb, in1=sin_sb[:, seg], op=OP.mult
                    )
                    for h in range(2):
                        r = 64 * h
                        nc.vector.tensor_tensor(
                            out=fold[32 * h : 32 * h + 32, 0, seg],
                            in0=qc[r : r + 32, :],
                            in1=qs[r + 32 : r + 64, :],
                            op=OP.subtract,
                        )
                        nc.vector.tensor_tensor(
                            out=fold[32 * h : 32 * h + 32, 1, seg],
                            in0=qs[r : r + 32, :],
                            in1=qc[r + 32 : r + 64, :],
                            op=OP.add,
                        )

                # k first, then v, then q: attention needs full k/v but q only
                # tile-by-tile.
                for xin, wsb, bsb, kind in (
                    (xk, wk_sb, bk_sb, "k"),
                    (xq, wq_sb, bq_sb, "q"),
                    (xv, wv_sb, bv_sb, "v"),
                ):
                    for gp in range(S // 1024):  # pairs of 512-col tiles
                        pjs = [
                            pjp.tile([P, TR], F32, tag="pj", name="pj")
                            for _ in range(2)
                        ]
                        engs = (nc.sync, nc.gpsimd)
                        for k in range(KC):
                            xt_c = xtp.tile([P, 1024], F32R, tag="xt", name="xt")
                            eng = engs[(gp * KC + k) % len(engs)]
                            eng.dma_start(
                                xt_c,
                                xin[b, k * P : (k + 1) * P, gp * 1024 : (gp + 1) * 1024],
                            )
                            for j in range(2):
                                nc.tensor.matmul(
                                    pjs[j],
                                    lhsT=wsb[:, k, :],
                                    rhs=r32(xt_c[:, j * TR : (j + 1) * TR]),
                                    start=(k == 0),
                                    stop=(k == KC - 1),
                                )
                            if k % 4 == 3:
                                yield
                        for j in range(2):
                            g = gp * 2 + j
                            seg = slice(g * TR, (g + 1) * TR)
                            if kind == "v":
                                nc.vector.tensor_scalar_add(vT[:, seg], pjs[j], bsb)
                            elif kind == "q":
                                rope(pjs[j], qfold, seg, bsb)
                            else:
                                rope(pjs[j], kfold, seg, bsb)
                        if kind == "v":
                            # transpose the two finished 512-col blocks into
                            # v1 [s, (h, d)] via DMA XBAR
                            for sc in range(gp * 8, gp * 8 + 8):
                                for h in range(2):
                                    nc.sync.dma_start(
                                        v1[:, sc, h, 0:64],
                                        vT[64 * h : 64 * h + 64, sc * P : (sc + 1) * P],
                                        transpose=True,
                                    )
                        yield

            def attn_stream(b):
                """Attention for batch b (heads sequential, t-ranges of 512)."""
                st = state[b]
                qfold, kfold, v1 = st["qfold"], st["kfold"], st["v1"]
                dv_sb = big.tile([P, SC, P], BF16, tag="dv", name="dv_sb")
                stg = big.tile([P, SC, P], BF16, tag="stg", name="stg")
                state[b]["stg"] = stg
                state[b]["dv_sb"] = dv_sb
                for h in range(2):
                    hp = slice(32 * h, 32 * h + 32)
                    for tr_i in range(NTR):
                        tseg = slice(tr_i * TR, (tr_i + 1) * TR)
                        acc = pavp.tile([P, NTS, P], F32, tag="acc", name="acc")
                        nc.gpsimd.memset(acc[:, :, 0:65], 0.0)
                        for sp in range(SC // 2):
                            psc = pscp.tile([P, 2, TR], F32, tag="psc", name="psc")
                            for j in range(2):
                                sc = 2 * sp + j
                                nc.tensor.matmul(
                                    psc[:, j, :],
                                    lhsT=kfold[hp, :, sc * P : (sc + 1) * P],
                                    rhs=qfold[hp, :, tseg],
                                    start=True,
                                    stop=True,
                                    perf_mode=DR,
                                )
                            pt = ptp.tile([P, 2, TR], BF16, tag="pt", name="pt")
                            nc.scalar.activation(pt, psc, AF.Exp, scale=0.125)
                            if DEBUG and b == 0 and h == 0 and tr_i == 0 and sp == 0:
                                nc.sync.dma_start(dbg["pt"][...], pt[:, :, :])
                            for j in range(2):
                                sc = 2 * sp + j
                                for ts in range(NTS):
                                    nc.tensor.matmul(
                                        acc[:, ts, 0:65],
                                        lhsT=pt[:, j, ts * P : (ts + 1) * P],
                                        rhs=v1[:, sc, h, :],
                                        start=False,
                                        stop=(sp == SC // 2 - 1 and j == 1),
                                        skip_group_check=True,
                                    )
                            yield
                        # epilogue: normalize into dv_sb
                        rc = tmp.tile([P, NTS, 1], F32, tag="rc", name="rc")
                        nc.vector.reciprocal(rc, acc[:, :, 64:65])
                        for ts in range(NTS):
                            nc.gpsimd.tensor_scalar_mul(
                                dv_sb[:, tr_i * NTS + ts, 64 * h : 64 * h + 64],
                                acc[:, ts, 0:64],
                                rc[:, ts, :],
                            )
                        if h == 1:
                            for ts in range(NTS):
                                tsg = tr_i * NTS + ts
                                nc.sync.dma_start(
                                    stg[:, tsg, :],
                                    dv_sb[:, tsg, :],
                                    transpose=True,
                                )
                        yield

            def tail(b, heads=(None,)):
                """Staging write + AllToAll + readback for batch b.

                heads=(None,) does one whole-batch AllToAll; heads=(0,)/(1,)
                do per-head AllToAlls so the first can fire mid-attention.
                """
                stg = state[b]["stg"]
                at_sb = state[b].get("at")
                if at_sb is None:
                    at_sb = iop.tile([P, KC, TS], BF16, tag="at", name="at")
                    state[b]["at"] = at_sb
                for h in heads:
                    hp = slice(0, P) if h is None else slice(64 * h, 64 * h + 64)
                    np_ = P if h is None else 64
                    a2a_in_b = dram.tile(
                        [N_CORES, np_, TS], BF16, tag=f"a2a_in{np_}", name="a2a_in"
                    )
                    a2a_out_b = dram.tile(
                        [N_CORES, np_, TS], BF16, tag=f"a2a_out{np_}", name="a2a_out"
                    )
                    nc.gpsimd.dma_start(
                        a2a_in_b.rearrange("c p t -> p c t"), stg[hp, :, :]
                    )
                    nc.gpsimd.collective_compute(
                        "AllToAll",
                        mybir.AluOpType.bypass,
                        replica_groups=[list(range(N_CORES))],
                        ins=[a2a_in_b.opt()],
                        outs=[a2a_out_b.opt()],
                    )
                    nc.sync.dma_start(
                        at_sb[hp, :, :], a2a_out_b.rearrange("c p t -> p c t")
                    )

            def outproj_stream(b):
                """Output projection for batch b's 256 rows (needs tail(b))."""
                at_sb = state[b]["at"]
                for n in range(D // TR):
                    nseg = slice(n * TR, (n + 1) * TR)
                    for m in range(TS // P):
                        py = pjp.tile([P, TR], F32, tag="pj", name="py")
                        for k in range(KC):
                            nc.tensor.matmul(
                                py,
                                lhsT=at_sb[:, k, m * P : (m + 1) * P],
                                rhs=wo_sb[:, k, nseg],
                                start=(k == 0),
                                stop=(k == KC - 1),
                            )
                        yo = tmp.tile([P, TR], F32, tag="yo", name="yo")
                        nc.vector.tensor_tensor(
                            out=yo, in0=py, in1=bo_sb[:, nseg], op=OP.add
                        )
                        nc.sync.dma_start(y[b, m * P : (m + 1) * P, nseg], yo)
                        yield

            def drain(gen, n=10**9):
                for _ in range(n):
                    try:
                        next(gen)
                    except StopIteration:
                        return True
                return False

            # ---- stage 1: batch-0 projections ----
            p0 = proj_stream(0)
            drain(p0)

            if DEBUG:
                st0 = state[0]
                nc.sync.dma_start(dbg["qfold"][...], st0["qfold"][:, :, :])
                nc.sync.dma_start(dbg["kfold"][...], st0["kfold"][:, :, :])
                nc.sync.dma_start(dbg["vT"][...], st0["vT"][:, :])
                nc.sync.dma_start(dbg["v1"][...], st0["v1"][:, :, :, :])

            # ---- stage 2: batch-0 attention interleaved with batch-1
            # projections ----
            a0 = attn_stream(0)
            p1 = proj_stream(1)
            i = 0
            while True:
                try:
                    next(a0)
                except StopIteration:
                    break
                if i % 2 == 0:
                    drain(p1, 1)
                i += 1
            drain(p1)

            if DEBUG:
                nc.sync.dma_start(dbg["dv"][...], state[0]["dv_sb"][:, :, :])
                nc.sync.dma_start(dbg["stg"][...], state[0]["stg"][:, :, :])

            # ---- stage 3: a2a for b0; batch-1 attention interleaved with
            # wo load + b0 output projection ----
            tail(0)
            if DEBUG:
                nc.sync.dma_start(dbg["at"][...], state[0]["at"][:, :, :])
            a1 = attn_stream(1)
            o0 = outproj_stream(0)
            i = 0
            while True:
                try:
                    next(a1)
                except StopIteration:
                    break
                if i == 2:
                    nc.sync.dma_start(
                        wo_sb, wo.rearrange("(ko p) m -> p ko m", p=P)
                    )
                if i >= 24 and i % 4 == 0:
                    drain(o0, 1)
                i += 1
            drain(o0)

            # ---- stage 4: a2a for b1 + b1 output projection ----
            tail(1)
            o1 = outproj_stream(1)
            drain(o1)

    nc.compile()
    return nc


def host_inputs(query, key_, value, Wq, bq, Wk, bk, Wv, bv, Wo, bo, S=2048):
    """Build per-core input maps (host-side sharding/layout prep)."""
    import ml_dtypes

    f = np.float32
    bf = ml_dtypes.bfloat16
    xq = np.ascontiguousarray(np.transpose(np.asarray(query, f), (0, 2, 1)))
    xk = np.ascontiguousarray(np.transpose(np.asarray(key_, f), (0, 2, 1)))
    xv = np.ascontiguousarray(np.transpose(np.asarray(value, f), (0, 2, 1)))
    wo_t = np.ascontiguousarray(np.asarray(Wo, f).T)
    bo_rep = np.ascontiguousarray(
        np.broadcast_to(np.asarray(bo, f)[None, :], (P, D_MODEL))
    )

    inv_freq = (
        1.0 / (ROPE_BASE ** (np.arange(0, HEAD_DIM, 2, dtype=f) / HEAD_DIM))
    ).astype(f)
    t = np.arange(S, dtype=f)
    freqs = np.einsum("i,j->ij", t, inv_freq).astype(f)  # [S, 32]
    emb = np.concatenate([freqs, freqs], axis=-1)  # [S, 64]
    cosT = np.cos(emb).astype(f).T  # [64, S]
    sinT = np.sin(emb).astype(f).T
    cos_rep = np.ascontiguousarray(np.tile(cosT, (2, 1))).astype(bf)  # [128, S]
    sin_rep = np.ascontiguousarray(np.tile(sinT, (2, 1))).astype(bf)

    Wq, Wk, Wv = (np.asarray(w, f) for w in (Wq, Wk, Wv))
    bq, bk, bv = (np.asarray(v_, f) for v_ in (bq, bk, bv))

    in_maps = []
    for c in range(N_CORES):
        sl = slice(P * c, P * (c + 1))
        in_maps.append(
            {
                "xq": xq,
                "xk": xk,
                "xv": xv,
                "wq": np.ascontiguousarray(Wq[sl, :].T),
                "wk": np.ascontiguousarray(Wk[sl, :].T),
                "wv": np.ascontiguousarray(Wv[sl, :].T),
                "bq": np.ascontiguousarray(bq[sl].reshape(P, 1)),
                "bk": np.ascontiguousarray(bk[sl].reshape(P, 1)),
                "bv": np.ascontiguousarray(bv[sl].reshape(P, 1)),
                "wo": wo_t,
                "bo": bo_rep,
                "cosr": cos_rep,
                "sinr": sin_rep,
            }
        )
    return in_maps


def kernel(query, key_, value, Wq, bq, Wk, bk, Wv, bv, Wo, bo):
    global LAST_RESULTS
    from concourse.bass_utils import run_bass_kernel_spmd

    S = query.shape[1]
    in_maps = host_inputs(
        query, key_, value, Wq, bq, Wk, bk, Wv, bv, Wo, bo, S=S
    )
    nc = build_nc(S=S)
    res = run_bass_kernel_spmd(
        nc, in_maps, core_ids=list(range(N_CORES)), trace=TRACE
    )
    LAST_RESULTS = res
    TS = S // N_CORES
    out = np.empty((B, S, D_MODEL), np.float32)
    for c in range(N_CORES):
        out[:, TS * c : TS * (c + 1), :] = res.results[c]["y"]
    return out


# revision 9
# speedup vs baseline: 1.1894x; 1.1245x over previous
"""Trainium2 Bass kernel for InterpretableMultiHeadAttention.

Sharding: 8 cores, head-parallel. Core c computes heads {2c, 2c+1} for both
batches (q/k/v projections sliced to 128 output features per core), runs full
attention for those heads, then one AllToAll per batch redistributes the
(feature-sharded) attention output into (row-sharded) form so each core
computes the final output projection for 256 rows of each batch.

Key layout/precision choices (driven by the instruction cost model):
  - Activations arrive pre-transposed from host: xT [B, D, S], so projections
    produce feature-major tiles directly.
  - RoPE on DVE in bf16; outputs written to a "folded" fp8 layout
    [32, 2, S] per head (d 0-31 in column block 0, d 32-63 in block 1) so the
    score matmuls can run in fp8 DoubleRow mode (0.5 cycles/row).
  - Scores psum [128 s, 2, 512 t]; exp on the scalar engine over 1024-wide
    tiles, output bf16.
  - AV in [t, d] orientation: accumulators [128 t, 4, 65] packed into one
    PSUM bank (memset + start=False accumulation), ones-column gives the
    softmax denominator; normalization via per-partition reciprocal scalars.
  - Layout transposes (v [d,s]->[s,d] and attention out [t,d]->[d,t]) use the
    DMA XBAR transpose (bf16), costing no PE/DVE/PSUM resources.
  - DMA load is split across the SP, Pool, and (when exp-idle) Activation
    queues; instruction emission interleaves the two batches so projections,
    attention, collectives, and the output projection overlap.
"""

import os
import sys

import numpy as np

sys.path.insert(0, "/opt/trn_rl_repo")

import concourse.bass as bass  # noqa: E402
from concourse import bacc  # noqa: E402
import concourse.tile as tile  # noqa: E402
from concourse import mybir  # noqa: E402

F32 = mybir.dt.float32
F32R = mybir.dt.float32r
BF16 = mybir.dt.bfloat16
FP8 = mybir.dt.float8e4
AF = mybir.ActivationFunctionType
OP = mybir.AluOpType
DR = mybir.MatmulPerfMode.DoubleRow

B = 2
D_MODEL = 1024
NHEAD = 16
HEAD_DIM = 64
N_CORES = 8
P = 128
ROPE_BASE = 10000.0

KC = D_MODEL // P  # 8 contraction chunks for projections
TR = 512  # attention t-range width (one psum pair tile)

# module globals so test.py can flip tracing and read timing
TRACE = bool(int(os.environ.get("BASS_KERNEL_TRACE", "0")))
DEBUG = bool(int(os.environ.get("BASS_KERNEL_DEBUG", "0")))
LAST_RESULTS = None


def r32(ap):
    if ap.dtype == F32R:
        return ap
    return ap.bitcast(F32R)


def build_nc(S=2048, T_TILE=None):
    """Build the SPMD program (identical on all 8 cores)."""
    del T_TILE
    D = D_MODEL
    SC = S // P  # 16 s chunks
    TS = S // N_CORES  # 256 rows per core per batch
    NTR = S // TR  # 4 t-ranges per head
    NTS = TR // P  # 4 t-subchunks per range

    nc = bacc.Bacc()

    xq = nc.declare_dram_parameter("xq", [B, D, S], F32R, isOutput=False)
    xk = nc.declare_dram_parameter("xk", [B, D, S], F32R, isOutput=False)
    xv = nc.declare_dram_parameter("xv", [B, D, S], F32R, isOutput=False)
    wq = nc.declare_dram_parameter("wq", [D, P], F32R, isOutput=False)
    wk = nc.declare_dram_parameter("wk", [D, P], F32R, isOutput=False)
    wv = nc.declare_dram_parameter("wv", [D, P], F32R, isOutput=False)
    bqp = nc.declare_dram_parameter("bq", [P, 1], F32, isOutput=False)
    bkp = nc.declare_dram_parameter("bk", [P, 1], F32, isOutput=False)
    bvp = nc.declare_dram_parameter("bv", [P, 1], F32, isOutput=False)
    wo = nc.declare_dram_parameter("wo", [D, D], F32R, isOutput=False)
    bop = nc.declare_dram_parameter("bo", [P, D], F32, isOutput=False)
    cosr = nc.declare_dram_parameter("cosr", [P, S], BF16, isOutput=False)
    sinr = nc.declare_dram_parameter("sinr", [P, S], BF16, isOutput=False)
    y = nc.declare_dram_parameter("y", [B, TS, D], F32, isOutput=True)
    dbg = {}
    if DEBUG:
        dbg["qfold"] = nc.declare_dram_parameter("dbg_qfold", [64, 2, S], FP8, isOutput=True)
        dbg["kfold"] = nc.declare_dram_parameter("dbg_kfold", [64, 2, S], FP8, isOutput=True)
        dbg["vT"] = nc.declare_dram_parameter("dbg_vT", [P, S], BF16, isOutput=True)
        dbg["v1"] = nc.declare_dram_parameter("dbg_v1", [P, SC, 2, 65], BF16, isOutput=True)
        dbg["pt"] = nc.declare_dram_parameter("dbg_pt", [P, 2, TR], BF16, isOutput=True)
        dbg["dv"] = nc.declare_dram_parameter("dbg_dv", [P, SC, P], BF16, isOutput=True)
        dbg["stg"] = nc.declare_dram_parameter("dbg_stg", [P, SC, P], BF16, isOutput=True)
        dbg["at"] = nc.declare_dram_parameter("dbg_at", [P, KC, TS], BF16, isOutput=True)

    with tile.TileContext(nc) as tc:
        with (
            nc.allow_low_precision(reason="fp8/bf16 attention pipeline"),
            tc.tile_pool(name="singles", bufs=1) as singles,
            tc.tile_pool(name="big", bufs=2) as big,
            tc.tile_pool(name="xt", bufs=8) as xtp,
            tc.tile_pool(name="tmp", bufs=3) as tmp,
            tc.tile_pool(name="pt", bufs=6) as ptp,
            tc.tile_pool(name="io", bufs=2) as iop,
            tc.tile_pool(name="pj", bufs=2, space="PSUM") as pjp,
            tc.tile_pool(name="qcs", bufs=1, space="PSUM") as qcsp,
            tc.tile_pool(name="psc", bufs=2, space="PSUM") as pscp,
            tc.tile_pool(name="pav", bufs=1, space="PSUM") as pavp,
            tc.tile_pool(name="dram", bufs=2, space="DRAM") as dram,
        ):
            # ---- persistent loads (weights etc. go on the ACT queue, which
            # is idle until the first exp) ----
            wq_sb = singles.tile([P, KC, P], F32R, tag="wq")
            wk_sb = singles.tile([P, KC, P], F32R, tag="wk")
            wv_sb = singles.tile([P, KC, P], F32R, tag="wv")
            nc.scalar.dma_start(wk_sb, wk.rearrange("(ko p) m -> p ko m", p=P))
            nc.scalar.dma_start(wv_sb, wv.rearrange("(ko p) m -> p ko m", p=P))
            nc.scalar.dma_start(wq_sb, wq.rearrange("(ko p) m -> p ko m", p=P))
            cos_sb = singles.tile([P, S], BF16, tag="cos")
            sin_sb = singles.tile([P, S], BF16, tag="sin")
            nc.scalar.dma_start(cos_sb, cosr[:, :])
            nc.scalar.dma_start(sin_sb, sinr[:, :])
            bq_sb = singles.tile([P, 1], F32, tag="bq")
            bk_sb = singles.tile([P, 1], F32, tag="bk")
            bv_sb = singles.tile([P, 1], F32, tag="bv")
            nc.sync.dma_start(bq_sb, bqp[:, :])
            nc.sync.dma_start(bk_sb, bkp[:, :])
            nc.sync.dma_start(bv_sb, bvp[:, :])
            bo_sb = singles.tile([P, D], F32, tag="bo")
            nc.scalar.dma_start(bo_sb, bop[:, :])
            wo_sb = singles.tile([P, KC, D], F32R, tag="wo")

            # per-batch state (bufs=2 ping-pongs across batches)
            state = {}

            def proj_stream(b):
                """Projections for batch b. Yields after small quanta."""
                vT = big.tile([P, S], BF16, tag="vT", name="vT")
                qfold = big.tile([64, 2, S], FP8, tag="qfold", name="qfold")
                kfold = big.tile([64, 2, S], FP8, tag="kfold", name="kfold")
                v1 = big.tile([P, SC, 2, 65], BF16, tag="v1", name="v1")
                nc.vector.memset(v1[:, :, :, 64:65], 1.0)
                state[b] = dict(vT=vT, qfold=qfold, kfold=kfold, v1=v1)

                def rope(pj, fold, seg, bias_sb):
                    qb = tmp.tile([P, TR], BF16, tag="qb", name="qb")
                    nc.vector.tensor_scalar_add(qb, pj, bias_sb)
                    qcs = qcsp.tile([P, 2, TR], BF16, tag="qcs", name="qcs")
                    qc = qcs[:, 0, :]
                    qs = qcs[:, 1, :]
                    nc.vector.tensor_tensor(
                        out=qc, in0=qb, in1=cos_sb[:, seg], op=OP.mult
                    )
                    nc.vector.tensor_tensor(
                        out=qs, in0=qb, in1=sin_sb[:, seg], op=OP.mult
                    )
                    for h in range(2):
                        r = 64 * h
                        nc.vector.tensor_tensor(
                            out=fold[32 * h : 32 * h + 32, 0, seg],
                            in0=qc[r : r + 32, :],
                            in1=qs[r + 32 : r + 64, :],
                            op=OP.subtract,
                        )
                        nc.vector.tensor_tensor(
                            out=fold[32 * h : 32 * h + 32, 1, seg],
                            in0=qs[r : r + 32, :],
                            in1=qc[r + 32 : r + 64, :],
                            op=OP.add,
                        )

                # k first, then v, then q: attention needs full k/v but q only
                # tile-by-tile.
                for gp in range(S // 1024):  # pairs of 512-col tiles
                    for xin, wsb, bsb, kind in (
                        (xk, wk_sb, bk_sb, "k"),
                        (xq, wq_sb, bq_sb, "q"),
                        (xv, wv_sb, bv_sb, "v"),
                    ):
                        pjs = [
                            pjp.tile([P, TR], F32, tag="pj", name="pj")
                            for _ in range(2)
                        ]
                        engs = (nc.sync, nc.gpsimd)
                        for k in range(KC):
                            xt_c = xtp.tile([P, 1024], F32R, tag="xt", name="xt")
                            eng = engs[(gp * KC + k) % len(engs)]
                            eng.dma_start(
                                xt_c,
                                xin[b, k * P : (k + 1) * P, gp * 1024 : (gp + 1) * 1024],
                            )
                            for j in range(2):
                                nc.tensor.matmul(
                                    pjs[j],
                                    lhsT=wsb[:, k, :],
                                    rhs=r32(xt_c[:, j * TR : (j + 1) * TR]),
                                    start=(k == 0),
                                    stop=(k == KC - 1),
                                )
                            if k % 4 == 3:
                                yield
                        for j in range(2):
                            g = gp * 2 + j
                            seg = slice(g * TR, (g + 1) * TR)
                            if kind == "v":
                                nc.vector.tensor_scalar_add(vT[:, seg], pjs[j], bsb)
                            elif kind == "q":
                                rope(pjs[j], qfold, seg, bsb)
                            else:
                                rope(pjs[j], kfold, seg, bsb)
                        if kind == "v":
                            # transpose the two finished 512-col blocks into
                            # v1 [s, (h, d)] via DMA XBAR
                            for sc in range(gp * 8, gp * 8 + 8):
                                for h in range(2):
                                    nc.sync.dma_start(
                                        v1[:, sc, h, 0:64],
                                        vT[64 * h : 64 * h + 64, sc * P : (sc + 1) * P],
                                        transpose=True,
                                    )
                        yield

            def attn_stream(b):
                """Attention for batch b (heads sequential, t-ranges of 512)."""
                st = state[b]
                qfold, kfold, v1 = st["qfold"], st["kfold"], st["v1"]
                veng = nc.gpsimd if b == 0 else nc.vector
                dv_sb = big.tile([P, SC, P], BF16, tag="dv", name="dv_sb")
                stg = big.tile([P, SC, P], BF16, tag="stg", name="stg")
                state[b]["stg"] = stg
                state[b]["dv_sb"] = dv_sb
                for h in range(2):
                    hp = slice(32 * h, 32 * h + 32)
                    for tr_i in range(NTR):
                        tseg = slice(tr_i * TR, (tr_i + 1) * TR)
                        acc = pavp.tile([P, NTS, P], F32, tag="acc", name="acc")
                        veng.memset(acc[:, :, 0:65], 0.0)
                        for sp in range(SC // 2):
                            psc = pscp.tile([P, 2, TR], F32, tag="psc", name="psc")
                            for j in range(2):
                                sc = 2 * sp + j
                                nc.tensor.matmul(
                                    psc[:, j, :],
                                    lhsT=kfold[hp, :, sc * P : (sc + 1) * P],
                                    rhs=qfold[hp, :, tseg],
                                    start=True,
                                    stop=True,
                                    perf_mode=DR,
                                )
                            pt = ptp.tile([P, 2, TR], BF16, tag="pt", name="pt")
                            nc.scalar.activation(pt, psc, AF.Exp, scale=0.125)
                            if DEBUG and b == 0 and h == 0 and tr_i == 0 and sp == 0:
                                nc.sync.dma_start(dbg["pt"][...], pt[:, :, :])
                            for j in range(2):
                                sc = 2 * sp + j
                                for ts in range(NTS):
                                    nc.tensor.matmul(
                                        acc[:, ts, 0:65],
                                        lhsT=pt[:, j, ts * P : (ts + 1) * P],
                                        rhs=v1[:, sc, h, :],
                                        start=False,
                                        stop=(sp == SC // 2 - 1 and j == 1),
                                        skip_group_check=True,
                                    )
                            yield
                        # epilogue: normalize into dv_sb
                        rc = tmp.tile([P, NTS, 1], F32, tag="rc", name="rc")
                        nc.vector.reciprocal(rc, acc[:, :, 64:65])
                        for ts in range(NTS):
                            veng.tensor_scalar_mul(
                                dv_sb[:, tr_i * NTS + ts, 64 * h : 64 * h + 64],
                                acc[:, ts, 0:64],
                                rc[:, ts, :],
                            )
                        if h == 1:
                            for ts in range(NTS):
                                tsg = tr_i * NTS + ts
                                nc.sync.dma_start(
                                    stg[:, tsg, :],
                                    dv_sb[:, tsg, :],
                                    transpose=True,
                                )
                        yield

            def tail(b, heads=(None,)):
                """Staging write + AllToAll + readback for batch b.

                heads=(None,) does one whole-batch AllToAll; heads=(0,)/(1,)
                do per-head AllToAlls so the first can fire mid-attention.
                """
                stg = state[b]["stg"]
                at_sb = state[b].get("at")
                if at_sb is None:
                    at_sb = iop.tile([P, KC, TS], BF16, tag="at", name="at")
                    state[b]["at"] = at_sb
                for h in heads:
                    hp = slice(0, P) if h is None else slice(64 * h, 64 * h + 64)
                    np_ = P if h is None else 64
                    a2a_in_b = dram.tile(
                        [N_CORES, np_, TS], BF16, tag=f"a2a_in{np_}", name="a2a_in"
                    )
                    a2a_out_b = dram.tile(
                        [N_CORES, np_, TS], BF16, tag=f"a2a_out{np_}", name="a2a_out"
                    )
                    nc.gpsimd.dma_start(
                        a2a_in_b.rearrange("c p t -> p c t"), stg[hp, :, :]
                    )
                    nc.gpsimd.collective_compute(
                        "AllToAll",
                        mybir.AluOpType.bypass,
                        replica_groups=[list(range(N_CORES))],
                        ins=[a2a_in_b.opt()],
                        outs=[a2a_out_b.opt()],
                    )
                    nc.sync.dma_start(
                        at_sb[hp, :, :], a2a_out_b.rearrange("c p t -> p c t")
                    )

            def outproj_stream(b):
                """Output projection for batch b's 256 rows (needs tail(b))."""
                at_sb = state[b]["at"]
                for n in range(D // TR):
                    nseg = slice(n * TR, (n + 1) * TR)
                    for m in range(TS // P):
                        py = pjp.tile([P, TR], F32, tag="pj", name="py")
                        for k in range(KC):
                            nc.tensor.matmul(
                                py,
                                lhsT=at_sb[:, k, m * P : (m + 1) * P],
                                rhs=wo_sb[:, k, nseg],
                                start=(k == 0),
                                stop=(k == KC - 1),
                            )
                        yo = tmp.tile([P, TR], F32, tag="yo", name="yo")
                        nc.vector.tensor_tensor(
                            out=yo, in0=py, in1=bo_sb[:, nseg], op=OP.add
                        )
                        nc.sync.dma_start(y[b, m * P : (m + 1) * P, nseg], yo)
                        yield

            def drain(gen, n=10**9):
                for _ in range(n):
                    try:
                        next(gen)
                    except StopIteration:
                        return True
                return False

            # ---- stage 1: batch-0 projections ----
            p0 = proj_stream(0)
            drain(p0)

            if DEBUG:
                st0 = state[0]
                nc.sync.dma_start(dbg["qfold"][...], st0["qfold"][:, :, :])
                nc.sync.dma_start(dbg["kfold"][...], st0["kfold"][:, :, :])
                nc.sync.dma_start(dbg["vT"][...], st0["vT"][:, :])
                nc.sync.dma_start(dbg["v1"][...], st0["v1"][:, :, :, :])

            # ---- stage 2: batch-0 attention interleaved with batch-1
            # projections ----
            a0 = attn_stream(0)
            p1 = proj_stream(1)
            i = 0
            while True:
                try:
                    next(a0)
                except StopIteration:
                    break
                if i % 2 == 0:
                    drain(p1, 1)
                i += 1
            drain(p1)

            if DEBUG:
                nc.sync.dma_start(dbg["dv"][...], state[0]["dv_sb"][:, :, :])
                nc.sync.dma_start(dbg["stg"][...], state[0]["stg"][:, :, :])

            # ---- stage 3: a2a for b0; batch-1 attention interleaved with
            # wo load + b0 output projection ----
            tail(0)
            if DEBUG:
                nc.sync.dma_start(dbg["at"][...], state[0]["at"][:, :, :])
            a1 = attn_stream(1)
            o0 = outproj_stream(0)
            i = 0
            while True:
                try:
                    next(a1)
                except StopIteration:
                    break
                if i == 2:
                    # WAW-gate the big wo load on a marker memset so the
                    # greedy scheduler cannot hoist it into the x-load phase
                    nc.vector.memset(wo_sb[:, :, 0:1], 0.0)
                    nc.sync.dma_start(
                        wo_sb, wo.rearrange("(ko p) m -> p ko m", p=P)
                    )
                if i >= 24 and i % 4 == 0:
                    drain(o0, 1)
                i += 1
            drain(o0)

            # ---- stage 4: a2a for b1 + b1 output projection ----
            tail(1)
            o1 = outproj_stream(1)
            drain(o1)

    nc.compile()
    return nc


def host_inputs(query, key_, value, Wq, bq, Wk, bk, Wv, bv, Wo, bo, S=2048):
    """Build per-core input maps (host-side sharding/layout prep)."""
    import ml_dtypes

    f = np.float32
    bf = ml_dtypes.bfloat16
    xq = np.ascontiguousarray(np.transpose(np.asarray(query, f), (0, 2, 1)))
    xk = np.ascontiguousarray(np.transpose(np.asarray(key_, f), (0, 2, 1)))
    xv = np.ascontiguousarray(np.transpose(np.asarray(value, f), (0, 2, 1)))
    wo_t = np.ascontiguousarray(np.asarray(Wo, f).T)
    bo_rep = np.ascontiguousarray(
        np.broadcast_to(np.asarray(bo, f)[None, :], (P, D_MODEL))
    )

    inv_freq = (
        1.0 / (ROPE_BASE ** (np.arange(0, HEAD_DIM, 2, dtype=f) / HEAD_DIM))
    ).astype(f)
    t = np.arange(S, dtype=f)
    freqs = np.einsum("i,j->ij", t, inv_freq).astype(f)  # [S, 32]
    emb = np.concatenate([freqs, freqs], axis=-1)  # [S, 64]
    cosT = np.cos(emb).astype(f).T  # [64, S]
    sinT = np.sin(emb).astype(f).T
    cos_rep = np.ascontiguousarray(np.tile(cosT, (2, 1))).astype(bf)  # [128, S]
    sin_rep = np.ascontiguousarray(np.tile(sinT, (2, 1))).astype(bf)

    Wq, Wk, Wv = (np.asarray(w, f) for w in (Wq, Wk, Wv))
    bq, bk, bv = (np.asarray(v_, f) for v_ in (bq, bk, bv))

    in_maps = []
    for c in range(N_CORES):
        sl = slice(P * c, P * (c + 1))
        in_maps.append(
            {
                "xq": xq,
                "xk": xk,
                "xv": xv,
                "wq": np.ascontiguousarray(Wq[sl, :].T),
                "wk": np.ascontiguousarray(Wk[sl, :].T),
                "wv": np.ascontiguousarray(Wv[sl, :].T),
                "bq": np.ascontiguousarray(bq[sl].reshape(P, 1)),
                "bk": np.ascontiguousarray(bk[sl].reshape(P, 1)),
                "bv": np.ascontiguousarray(bv[sl].reshape(P, 1)),
                "wo": wo_t,
                "bo": bo_rep,
                "cosr": cos_rep,
                "sinr": sin_rep,
            }
        )
    return in_maps


def kernel(query, key_, value, Wq, bq, Wk, bk, Wv, bv, Wo, bo):
    global LAST_RESULTS
    from concourse.bass_utils import run_bass_kernel_spmd

    S = query.shape[1]
    in_maps = host_inputs(
        query, key_, value, Wq, bq, Wk, bk, Wv, bv, Wo, bo, S=S
    )
    nc = build_nc(S=S)
    res = run_bass_kernel_spmd(
        nc, in_maps, core_ids=list(range(N_CORES)), trace=TRACE
    )
    LAST_RESULTS = res
    TS = S // N_CORES
    out = np.empty((B, S, D_MODEL), np.float32)
    for c in range(N_CORES):
        out[:, TS * c : TS * (c + 1), :] = res.results[c]["y"]
    return out


# revision 10
# speedup vs baseline: 1.1916x; 1.0019x over previous
"""Trainium2 Bass kernel for InterpretableMultiHeadAttention.

Sharding: 8 cores, head-parallel. Core c computes heads {2c, 2c+1} for both
batches (q/k/v projections sliced to 128 output features per core), runs full
attention for those heads, then one AllToAll per batch redistributes the
(feature-sharded) attention output into (row-sharded) form so each core
computes the final output projection for 256 rows of each batch.

Key layout/precision choices (driven by the instruction cost model):
  - Activations arrive pre-transposed from host: xT [B, D, S], so projections
    produce feature-major tiles directly.
  - RoPE on DVE in bf16; outputs written to a "folded" fp8 layout
    [32, 2, S] per head (d 0-31 in column block 0, d 32-63 in block 1) so the
    score matmuls can run in fp8 DoubleRow mode (0.5 cycles/row).
  - Scores psum [128 s, 2, 512 t]; exp on the scalar engine over 1024-wide
    tiles, output bf16.
  - AV in [t, d] orientation: accumulators [128 t, 4, 65] packed into one
    PSUM bank (memset + start=False accumulation), ones-column gives the
    softmax denominator; normalization via per-partition reciprocal scalars.
  - Layout transposes (v [d,s]->[s,d] and attention out [t,d]->[d,t]) use the
    DMA XBAR transpose (bf16), costing no PE/DVE/PSUM resources.
  - DMA load is split across the SP, Pool, and (when exp-idle) Activation
    queues; instruction emission interleaves the two batches so projections,
    attention, collectives, and the output projection overlap.
"""

import os
import sys

import numpy as np

sys.path.insert(0, "/opt/trn_rl_repo")

import concourse.bass as bass  # noqa: E402
from concourse import bacc  # noqa: E402
import concourse.tile as tile  # noqa: E402
from concourse import mybir  # noqa: E402

F32 = mybir.dt.float32
F32R = mybir.dt.float32r
BF16 = mybir.dt.bfloat16
FP8 = mybir.dt.float8e4
AF = mybir.ActivationFunctionType
OP = mybir.AluOpType
DR = mybir.MatmulPerfMode.DoubleRow

B = 2
D_MODEL = 1024
NHEAD = 16
HEAD_DIM = 64
N_CORES = 8
P = 128
ROPE_BASE = 10000.0

KC = D_MODEL // P  # 8 contraction chunks for projections
TR = 512  # attention t-range width (one psum pair tile)

# module globals so test.py can flip tracing and read timing
TRACE = bool(int(os.environ.get("BASS_KERNEL_TRACE", "0")))
DEBUG = bool(int(os.environ.get("BASS_KERNEL_DEBUG", "0")))
LAST_RESULTS = None


def r32(ap):
    if ap.dtype == F32R:
        return ap
    return ap.bitcast(F32R)


def build_nc(S=2048, T_TILE=None):
    """Build the SPMD program (identical on all 8 cores)."""
    del T_TILE
    D = D_MODEL
    SC = S // P  # 16 s chunks
    TS = S // N_CORES  # 256 rows per core per batch
    NTR = S // TR  # 4 t-ranges per head
    NTS = TR // P  # 4 t-subchunks per range

    nc = bacc.Bacc()

    xq = nc.declare_dram_parameter("xq", [B, D, S], F32R, isOutput=False)
    xk = nc.declare_dram_parameter("xk", [B, D, S], F32R, isOutput=False)
    xv = nc.declare_dram_parameter("xv", [B, D, S], F32R, isOutput=False)
    wq = nc.declare_dram_parameter("wq", [D, P], F32R, isOutput=False)
    wk = nc.declare_dram_parameter("wk", [D, P], F32R, isOutput=False)
    wv = nc.declare_dram_parameter("wv", [D, P], F32R, isOutput=False)
    bqp = nc.declare_dram_parameter("bq", [P, 1], F32, isOutput=False)
    bkp = nc.declare_dram_parameter("bk", [P, 1], F32, isOutput=False)
    bvp = nc.declare_dram_parameter("bv", [P, 1], F32, isOutput=False)
    wo = nc.declare_dram_parameter("wo", [D, D], F32R, isOutput=False)
    bop = nc.declare_dram_parameter("bo", [P, D], F32, isOutput=False)
    cosr = nc.declare_dram_parameter("cosr", [P, S], BF16, isOutput=False)
    sinr = nc.declare_dram_parameter("sinr", [P, S], BF16, isOutput=False)
    y = nc.declare_dram_parameter("y", [B, TS, D], F32, isOutput=True)
    dbg = {}
    if DEBUG:
        dbg["qfold"] = nc.declare_dram_parameter("dbg_qfold", [64, 2, S], FP8, isOutput=True)
        dbg["kfold"] = nc.declare_dram_parameter("dbg_kfold", [64, 2, S], FP8, isOutput=True)
        dbg["vT"] = nc.declare_dram_parameter("dbg_vT", [P, S], BF16, isOutput=True)
        dbg["v1"] = nc.declare_dram_parameter("dbg_v1", [P, SC, 2, 65], BF16, isOutput=True)
        dbg["pt"] = nc.declare_dram_parameter("dbg_pt", [P, 2, TR], BF16, isOutput=True)
        dbg["dv"] = nc.declare_dram_parameter("dbg_dv", [P, SC, P], BF16, isOutput=True)
        dbg["stg"] = nc.declare_dram_parameter("dbg_stg", [P, SC, P], BF16, isOutput=True)
        dbg["at"] = nc.declare_dram_parameter("dbg_at", [P, KC, TS], BF16, isOutput=True)

    with tile.TileContext(nc) as tc:
        with (
            nc.allow_low_precision(reason="fp8/bf16 attention pipeline"),
            tc.tile_pool(name="singles", bufs=1) as singles,
            tc.tile_pool(name="big", bufs=2) as big,
            tc.tile_pool(name="xt", bufs=8) as xtp,
            tc.tile_pool(name="tmp", bufs=3) as tmp,
            tc.tile_pool(name="pt", bufs=6) as ptp,
            tc.tile_pool(name="io", bufs=2) as iop,
            tc.tile_pool(name="pj", bufs=2, space="PSUM") as pjp,
            tc.tile_pool(name="qcs", bufs=1, space="PSUM") as qcsp,
            tc.tile_pool(name="psc", bufs=2, space="PSUM") as pscp,
            tc.tile_pool(name="pav", bufs=1, space="PSUM") as pavp,
            tc.tile_pool(name="dram", bufs=2, space="DRAM") as dram,
        ):
            # ---- persistent loads (weights etc. go on the ACT queue, which
            # is idle until the first exp) ----
            wq_sb = singles.tile([P, KC, P], F32R, tag="wq")
            wk_sb = singles.tile([P, KC, P], F32R, tag="wk")
            wv_sb = singles.tile([P, KC, P], F32R, tag="wv")
            nc.scalar.dma_start(wk_sb, wk.rearrange("(ko p) m -> p ko m", p=P))
            nc.scalar.dma_start(wv_sb, wv.rearrange("(ko p) m -> p ko m", p=P))
            nc.scalar.dma_start(wq_sb, wq.rearrange("(ko p) m -> p ko m", p=P))
            cos_sb = singles.tile([P, S], BF16, tag="cos")
            sin_sb = singles.tile([P, S], BF16, tag="sin")
            nc.scalar.dma_start(cos_sb, cosr[:, :])
            nc.scalar.dma_start(sin_sb, sinr[:, :])
            bq_sb = singles.tile([P, 1], F32, tag="bq")
            bk_sb = singles.tile([P, 1], F32, tag="bk")
            bv_sb = singles.tile([P, 1], F32, tag="bv")
            nc.sync.dma_start(bq_sb, bqp[:, :])
            nc.sync.dma_start(bk_sb, bkp[:, :])
            nc.sync.dma_start(bv_sb, bvp[:, :])
            bo_sb = singles.tile([P, D], F32, tag="bo")
            nc.scalar.dma_start(bo_sb, bop[:, :])
            wo_sb = singles.tile([P, KC, D], F32R, tag="wo")

            # per-batch state (bufs=2 ping-pongs across batches)
            state = {}

            def proj_stream(b):
                """Projections for batch b. Yields after small quanta."""
                vT = big.tile([P, S], BF16, tag="vT", name="vT")
                qfold = big.tile([64, 2, S], FP8, tag="qfold", name="qfold")
                kfold = big.tile([64, 2, S], FP8, tag="kfold", name="kfold")
                v1 = big.tile([P, SC, 2, 65], BF16, tag="v1", name="v1")
                nc.vector.memset(v1[:, :, :, 64:65], 1.0)
                state[b] = dict(vT=vT, qfold=qfold, kfold=kfold, v1=v1)

                def rope(pj, fold, seg, bias_sb):
                    qb = tmp.tile([P, TR], BF16, tag="qb", name="qb")
                    nc.vector.tensor_scalar_add(qb, pj, bias_sb)
                    qcs = qcsp.tile([P, 2, TR], BF16, tag="qcs", name="qcs")
                    qc = qcs[:, 0, :]
                    qs = qcs[:, 1, :]
                    nc.vector.tensor_tensor(
                        out=qc, in0=qb, in1=cos_sb[:, seg], op=OP.mult
                    )
                    nc.vector.tensor_tensor(
                        out=qs, in0=qb, in1=sin_sb[:, seg], op=OP.mult
                    )
                    for h in range(2):
                        r = 64 * h
                        nc.vector.tensor_tensor(
                            out=fold[32 * h : 32 * h + 32, 0, seg],
                            in0=qc[r : r + 32, :],
                            in1=qs[r + 32 : r + 64, :],
                            op=OP.subtract,
                        )
                        nc.vector.tensor_tensor(
                            out=fold[32 * h : 32 * h + 32, 1, seg],
                            in0=qs[r : r + 32, :],
                            in1=qc[r + 32 : r + 64, :],
                            op=OP.add,
                        )

                # k first, then v, then q: attention needs full k/v but q only
                # tile-by-tile.
                for gp in range(S // 1024):  # pairs of 512-col tiles
                    for xin, wsb, bsb, kind in (
                        (xk, wk_sb, bk_sb, "k"),
                        (xq, wq_sb, bq_sb, "q"),
                        (xv, wv_sb, bv_sb, "v"),
                    ):
                        pjs = [
                            pjp.tile([P, TR], F32, tag="pj", name="pj")
                            for _ in range(2)
                        ]
                        engs = (nc.sync, nc.gpsimd)
                        for k in range(KC):
                            xt_c = xtp.tile([P, 1024], F32R, tag="xt", name="xt")
                            eng = engs[(gp * KC + k) % len(engs)]
                            eng.dma_start(
                                xt_c,
                                xin[b, k * P : (k + 1) * P, gp * 1024 : (gp + 1) * 1024],
                            )
                            for j in range(2):
                                nc.tensor.matmul(
                                    pjs[j],
                                    lhsT=wsb[:, k, :],
                                    rhs=r32(xt_c[:, j * TR : (j + 1) * TR]),
                                    start=(k == 0),
                                    stop=(k == KC - 1),
                                )
                            if k % 4 == 3:
                                yield
                        for j in range(2):
                            g = gp * 2 + j
                            seg = slice(g * TR, (g + 1) * TR)
                            if kind == "v":
                                nc.vector.tensor_scalar_add(vT[:, seg], pjs[j], bsb)
                            elif kind == "q":
                                rope(pjs[j], qfold, seg, bsb)
                            else:
                                rope(pjs[j], kfold, seg, bsb)
                        if kind == "v":
                            # transpose the two finished 512-col blocks into
                            # v1 [s, (h, d)] via DMA XBAR
                            for sc in range(gp * 8, gp * 8 + 8):
                                for h in range(2):
                                    nc.sync.dma_start(
                                        v1[:, sc, h, 0:64],
                                        vT[64 * h : 64 * h + 64, sc * P : (sc + 1) * P],
                                        transpose=True,
                                    )
                        yield

            def attn_stream(b):
                """Attention for batch b (heads sequential, t-ranges of 512)."""
                st = state[b]
                qfold, kfold, v1 = st["qfold"], st["kfold"], st["v1"]
                veng = nc.gpsimd if b == 0 else nc.vector
                dv_sb = big.tile([P, SC, P], BF16, tag="dv", name="dv_sb")
                stg = big.tile([P, SC, P], BF16, tag="stg", name="stg")
                state[b]["stg"] = stg
                state[b]["dv_sb"] = dv_sb
                for h in range(2):
                    hp = slice(32 * h, 32 * h + 32)
                    for tr_i in range(NTR):
                        tseg = slice(tr_i * TR, (tr_i + 1) * TR)
                        acc = pavp.tile([P, NTS, P], F32, tag="acc", name="acc")
                        veng.memset(acc[:, :, 0:65], 0.0)
                        for sp in range(SC // 2):
                            psc = pscp.tile([P, 2, TR], F32, tag="psc", name="psc")
                            for j in range(2):
                                sc = 2 * sp + j
                                nc.tensor.matmul(
                                    psc[:, j, :],
                                    lhsT=kfold[hp, :, sc * P : (sc + 1) * P],
                                    rhs=qfold[hp, :, tseg],
                                    start=True,
                                    stop=True,
                                    perf_mode=DR,
                                )
                            pt = ptp.tile([P, 2, TR], BF16, tag="pt", name="pt")
                            nc.scalar.activation(pt, psc, AF.Exp, scale=0.125)
                            if DEBUG and b == 0 and h == 0 and tr_i == 0 and sp == 0:
                                nc.sync.dma_start(dbg["pt"][...], pt[:, :, :])
                            for j in range(2):
                                sc = 2 * sp + j
                                for ts in range(NTS):
                                    nc.tensor.matmul(
                                        acc[:, ts, 0:65],
                                        lhsT=pt[:, j, ts * P : (ts + 1) * P],
                                        rhs=v1[:, sc, h, :],
                                        start=False,
                                        stop=(sp == SC // 2 - 1 and j == 1),
                                        skip_group_check=True,
                                    )
                            yield
                        # epilogue: normalize into dv_sb
                        rc = tmp.tile([P, NTS, 1], F32, tag="rc", name="rc")
                        nc.vector.reciprocal(rc, acc[:, :, 64:65])
                        for ts in range(NTS):
                            veng.tensor_scalar_mul(
                                dv_sb[:, tr_i * NTS + ts, 64 * h : 64 * h + 64],
                                acc[:, ts, 0:64],
                                rc[:, ts, :],
                            )
                        if h == 1:
                            for ts in range(NTS):
                                tsg = tr_i * NTS + ts
                                nc.sync.dma_start(
                                    stg[:, tsg, :],
                                    dv_sb[:, tsg, :],
                                    transpose=True,
                                )
                        yield

            def tail(b, heads=(None,)):
                """Staging write + AllToAll + readback for batch b.

                heads=(None,) does one whole-batch AllToAll; heads=(0,)/(1,)
                do per-head AllToAlls so the first can fire mid-attention.
                """
                stg = state[b]["stg"]
                at_sb = state[b].get("at")
                if at_sb is None:
                    at_sb = iop.tile([P, KC, TS], BF16, tag="at", name="at")
                    state[b]["at"] = at_sb
                for h in heads:
                    hp = slice(0, P) if h is None else slice(64 * h, 64 * h + 64)
                    np_ = P if h is None else 64
                    a2a_in_b = dram.tile(
                        [N_CORES, np_, TS], BF16, tag=f"a2a_in{np_}", name="a2a_in"
                    )
                    a2a_out_b = dram.tile(
                        [N_CORES, np_, TS], BF16, tag=f"a2a_out{np_}", name="a2a_out"
                    )
                    nc.gpsimd.dma_start(
                        a2a_in_b.rearrange("c p t -> p c t"), stg[hp, :, :]
                    )
                    nc.gpsimd.collective_compute(
                        "AllToAll",
                        mybir.AluOpType.bypass,
                        replica_groups=[list(range(N_CORES))],
                        ins=[a2a_in_b.opt()],
                        outs=[a2a_out_b.opt()],
                    )
                    nc.sync.dma_start(
                        at_sb[hp, 0:4, :],
                        a2a_out_b.rearrange("c p t -> p c t")[:, 0:4, :],
                    )
                    nc.gpsimd.dma_start(
                        at_sb[hp, 4:8, :],
                        a2a_out_b.rearrange("c p t -> p c t")[:, 4:8, :],
                    )

            def outproj_stream(b):
                """Output projection for batch b's 256 rows (needs tail(b))."""
                at_sb = state[b]["at"]
                for n in range(D // TR):
                    nseg = slice(n * TR, (n + 1) * TR)
                    for m in range(TS // P):
                        py = pjp.tile([P, TR], F32, tag="pj", name="py")
                        for k in range(KC):
                            nc.tensor.matmul(
                                py,
                                lhsT=at_sb[:, k, m * P : (m + 1) * P],
                                rhs=wo_sb[:, k, nseg],
                                start=(k == 0),
                                stop=(k == KC - 1),
                            )
                        yo = tmp.tile([P, TR], F32, tag="yo", name="yo")
                        nc.vector.tensor_tensor(
                            out=yo, in0=py, in1=bo_sb[:, nseg], op=OP.add
                        )
                        yeng = nc.sync if (n + m) % 2 == 0 else nc.gpsimd
                        yeng.dma_start(y[b, m * P : (m + 1) * P, nseg], yo)
                        yield

            def drain(gen, n=10**9):
                for _ in range(n):
                    try:
                        next(gen)
                    except StopIteration:
                        return True
                return False

            # ---- stage 1: batch-0 projections ----
            p0 = proj_stream(0)
            drain(p0)

            if DEBUG:
                st0 = state[0]
                nc.sync.dma_start(dbg["qfold"][...], st0["qfold"][:, :, :])
                nc.sync.dma_start(dbg["kfold"][...], st0["kfold"][:, :, :])
                nc.sync.dma_start(dbg["vT"][...], st0["vT"][:, :])
                nc.sync.dma_start(dbg["v1"][...], st0["v1"][:, :, :, :])

            # ---- stage 2: batch-0 attention interleaved with batch-1
            # projections ----
            a0 = attn_stream(0)
            p1 = proj_stream(1)
            i = 0
            while True:
                try:
                    next(a0)
                except StopIteration:
                    break
                if i % 2 == 0:
                    drain(p1, 1)
                i += 1
            drain(p1)

            if DEBUG:
                nc.sync.dma_start(dbg["dv"][...], state[0]["dv_sb"][:, :, :])
                nc.sync.dma_start(dbg["stg"][...], state[0]["stg"][:, :, :])

            # ---- stage 3: a2a for b0; batch-1 attention interleaved with
            # wo load + b0 output projection ----
            tail(0)
            if DEBUG:
                nc.sync.dma_start(dbg["at"][...], state[0]["at"][:, :, :])
            a1 = attn_stream(1)
            o0 = outproj_stream(0)
            i = 0
            while True:
                try:
                    next(a1)
                except StopIteration:
                    break
                if i == 2:
                    # WAW-gate the big wo load on a marker memset so the
                    # greedy scheduler cannot hoist it into the x-load phase
                    nc.vector.memset(wo_sb[:, :, 0:1], 0.0)
                    nc.sync.dma_start(
                        wo_sb, wo.rearrange("(ko p) m -> p ko m", p=P)
                    )
                if i >= 24 and i % 4 == 0:
                    drain(o0, 1)
                i += 1
            drain(o0)

            # ---- stage 4: a2a for b1 + b1 output projection ----
            tail(1)
            o1 = outproj_stream(1)
            drain(o1)

    nc.compile()
    return nc


def host_inputs(query, key_, value, Wq, bq, Wk, bk, Wv, bv, Wo, bo, S=2048):
    """Build per-core input maps (host-side sharding/layout prep)."""
    import ml_dtypes

    f = np.float32
    bf = ml_dtypes.bfloat16
    xq = np.ascontiguousarray(np.transpose(np.asarray(query, f), (0, 2, 1)))
    xk = np.ascontiguousarray(np.transpose(np.asarray(key_, f), (0, 2, 1)))
    xv = np.ascontiguousarray(np.transpose(np.asarray(value, f), (0, 2, 1)))
    wo_t = np.ascontiguousarray(np.asarray(Wo, f).T)
    bo_rep = np.ascontiguousarray(
        np.broadcast_to(np.asarray(bo, f)[None, :], (P, D_MODEL))
    )

    inv_freq = (
        1.0 / (ROPE_BASE ** (np.arange(0, HEAD_DIM, 2, dtype=f) / HEAD_DIM))
    ).astype(f)
    t = np.arange(S, dtype=f)
    freqs = np.einsum("i,j->ij", t, inv_freq).astype(f)  # [S, 32]
    emb = np.concatenate([freqs, freqs], axis=-1)  # [S, 64]
    cosT = np.cos(emb).astype(f).T  # [64, S]
    sinT = np.sin(emb).astype(f).T
    cos_rep = np.ascontiguousarray(np.tile(cosT, (2, 1))).astype(bf)  # [128, S]
    sin_rep = np.ascontiguousarray(np.tile(sinT, (2, 1))).astype(bf)

    Wq, Wk, Wv = (np.asarray(w, f) for w in (Wq, Wk, Wv))
    bq, bk, bv = (np.asarray(v_, f) for v_ in (bq, bk, bv))

    in_maps = []
    for c in range(N_CORES):
        sl = slice(P * c, P * (c + 1))
        in_maps.append(
            {
                "xq": xq,
                "xk": xk,
                "xv": xv,
                "wq": np.ascontiguousarray(Wq[sl, :].T),
                "wk": np.ascontiguousarray(Wk[sl, :].T),
                "wv": np.ascontiguousarray(Wv[sl, :].T),
                "bq": np.ascontiguousarray(bq[sl].reshape(P, 1)),
                "bk": np.ascontiguousarray(bk[sl].reshape(P, 1)),
                "bv": np.ascontiguousarray(bv[sl].reshape(P, 1)),
                "wo": wo_t,
                "bo": bo_rep,
                "cosr": cos_rep,
                "sinr": sin_rep,
            }
        )
    return in_maps


def kernel(query, key_, value, Wq, bq, Wk, bk, Wv, bv, Wo, bo):
    global LAST_RESULTS
    from concourse.bass_utils import run_bass_kernel_spmd

    S = query.shape[1]
    in_maps = host_inputs(
        query, key_, value, Wq, bq, Wk, bk, Wv, bv, Wo, bo, S=S
    )
    nc = build_nc(S=S)
    res = run_bass_kernel_spmd(
        nc, in_maps, core_ids=list(range(N_CORES)), trace=TRACE
    )
    LAST_RESULTS = res
    TS = S // N_CORES
    out = np.empty((B, S, D_MODEL), np.float32)
    for c in range(N_CORES):
        out[:, TS * c : TS * (c + 1), :] = res.results[c]["y"]
    return out
